# revision 19
# baseline (speedup 1.0000x reference)
"""Trainium2 Bass kernel for the NodeAttentionLayer (GAT-style) problem, v2.

Math (per reference):
    h_t = t @ W_t; h_o = o @ W_o; s_t = h_t a_t; s_o = h_o a_o
    att = softmax(where(adj>0, lrelu(s_t[i]+s_o[j]), -inf), axis=j)
    out = elu(att @ h_o)

Identity used: exp(lrelu(y)) = max(exp(y), exp(0.2 y)).  In units of
u2_i = exp(0.2 s_t[i]) (cancels in the softmax ratio), with
    p_i = exp(0.8 s_t[i]),  v1_j = exp(s_o[j]),  v2_j = exp(0.2 s_o[j]):
    W[j,i] = adj_ij * max(p_i v1_j, v2_j)
Numerator and denominator together: Num[f,i] = sum_j hoext[j,f] W[j,i],
hoext = [h_o | 1].  Per j-tile of 128:
    X = ts(p_bcast, v1, v2, mult, max)      (one DVE op)
    V = X * adj                             (DVE or gpsimd)
    acc[f, i] += hoext_tile^T @ V           (PSUM accumulation)
Final: out = elu(Num[0:64] / Num[64]).  No compare, no exp in the loop,
no per-element softmax pass.

Sharding: rows of t_input/adj (N_t) across 8 cores; o replicated.
Output computed transposed [F, rows]; host transposes back.
"""

import contextlib
import ctypes
import sys
import types

import ml_dtypes
import numpy as np

import concourse.bass as bass
import concourse.mybir as mybir
import concourse.tile as tile
from concourse.vector_clock import ScopedClock

bf16 = ml_dtypes.bfloat16

# ---------------------------------------------------------------------------
# Environment shims
# ---------------------------------------------------------------------------

def _patch_tile_drain():
    """walrus in this container allows only one sync-wait per sync-engine
    instruction; split the TileContext epilogue drain's waits onto
    individual nops."""
    if getattr(tile.TileContext, "_drain_patch_installed", False):
        return

    def _drain_and_barrier(self, tick_clock, wait_clock):
        nop_inst = self.nc.sync.nop(nofuse=True)
        wait_clock.add_sem_waits(
            nop_inst.ins, ScopedClock({None: tick_clock.global_clock})
        )
        ow = list(nop_inst.ins.sync_info.on_wait) if nop_inst.ins.sync_info else []
        if len(ow) > 1:
            nop_inst.ins.sync_info.on_wait = ow[:1]
            for w in ow[1:]:
                extra = self.nc.sync.nop(nofuse=True)
                if extra.ins.sync_info is None:
                    extra.ins.sync_info = mybir.SyncInfo(on_wait=[w], on_update=[])
                else:
                    extra.ins.sync_info.on_wait = [w]
        self.nc.sync.drain()
        self.nc.all_engine_barrier()
        popped = self.nc._tile_sem_poison_stack.pop()
        assert popped is self._sem_poison
        self.nc.clear_and_free_semaphores(list(self.sems.allocated().values()))
        self.nc.all_engine_barrier()

    tile.TileContext._drain_and_barrier = _drain_and_barrier
    tile.TileContext._drain_patch_installed = True


def _install_ntff_hook():
    """Provide antenv.axon_hooks (absent in this image) so trace=True works."""
    if "antenv.axon_hooks" in sys.modules:
        return
    import antenv

    state = {"hook": None}
    mod = types.ModuleType("antenv.axon_hooks")
    mod.set_axon_ntff_profile_hook = lambda h: state.__setitem__("hook", h)
    mod.get_axon_ntff_profile_hook = lambda: state["hook"]
    sys.modules["antenv.axon_hooks"] = mod
    antenv.axon_hooks = mod

    try:
        lib = ctypes.CDLL("/opt/axon/libaxon_pjrt.so")
    except OSError:
        return
    if not hasattr(lib, "axon_start_nrt_profile"):
        return
    lib.axon_start_nrt_profile.argtypes = [
        ctypes.POINTER(ctypes.c_int64),
        ctypes.c_size_t,
    ]
    lib.axon_start_nrt_profile.restype = ctypes.c_int64
    lib.axon_stop_nrt_profile.argtypes = [ctypes.c_char_p]
    lib.axon_stop_nrt_profile.restype = ctypes.c_int64

    @contextlib.contextmanager
    def _ntff_hook(output_dir, device_ids):
        import jax

        jax.devices()
        if device_ids:
            ids = (ctypes.c_int64 * len(device_ids))(*device_ids)
            rc = lib.axon_start_nrt_profile(ids, len(device_ids))
        else:
            rc = lib.axon_start_nrt_profile(None, 0)
        if rc != 0:
            raise RuntimeError(f"axon_start_nrt_profile rc={rc}")
        try:
            yield
        finally:
            n = lib.axon_stop_nrt_profile(str(output_dir).encode())
            print(f"profile: {n} file(s) written to {output_dir}", file=sys.stderr)

    state["hook"] = _ntff_hook


_patch_tile_drain()
_install_ntff_hook()


def _split_multi_waits(nc):
    """walrus here accepts at most ONE sync-wait per instruction; hoist extra
    waits onto same-engine nops inserted immediately before."""
    import bass_rust

    k = 0
    for f in nc.m.functions:
        for blk in f.blocks:
            insts = blk.instructions
            out = []
            changed = False
            for inst in insts:
                si = inst.sync_info
                ow = list(si.on_wait) if si is not None else []
                if len(ow) > 1:
                    for w in ow[:-1]:
                        nop = bass_rust.InstNoOp(
                            name=f"waitsplit-{k}", engine=inst.engine
                        )
                        k += 1
                        nop.sync_info = mybir.SyncInfo(on_wait=[w], on_update=[])
                        out.append(nop)
                    si.on_wait = [ow[-1]]
                    changed = True
                out.append(inst)
            if changed:
                blk.instructions = out

# ---------------------------------------------------------------------------
# Problem constants (hardcoded per spec)
# ---------------------------------------------------------------------------
N_T, N_O, F_IN, F_OUT = 8192, 8192, 256, 64
N_CORES = 8
R = N_T // N_CORES            # rows (i) per core = 1024
NJ = N_O // 128               # j tiles of 128 = 64
KC = F_IN // 128              # contraction chunks for projections = 2
GROUP = 16                    # j-tiles per setup group
NG = NJ // GROUP              # setup groups = 4
FE = F_OUT + 1                # 65: h_o columns + ones column
NI2 = R // 512                # psum column chunks = 2
F32 = mybir.dt.float32
BF16 = mybir.dt.bfloat16
AF = mybir.ActivationFunctionType
OP = mybir.AluOpType

# Per-tile strategy:
#  'b' = DVE ts(mult,max) X + DVE tt mult
#  'a' = scalar-engine relu U + mult (gpsimd or DVE) + Term1 matmul on PE
TILE_MODE = ['a' if t % 8 in (2, 5, 7) else 'b' for t in range(NJ)]
# mask-mult engine for 'a' tiles
MULT_ENG = ['g' if t % 8 in (2, 7) else 'v' for t in range(NJ)]

def build_kernel(split_waits=True):
    nc = bass.Bass("TRN2")

    t_T = nc.dram_tensor("t_T", [F_IN, R], BF16, kind="ExternalInput")
    o_T = nc.dram_tensor("o_T", [F_IN, N_O], BF16, kind="ExternalInput")
    w_t = nc.dram_tensor("w_t", [F_IN, F_OUT], F32, kind="ExternalInput")
    w_o = nc.dram_tensor("w_o", [F_IN, F_OUT], F32, kind="ExternalInput")
    a_vec = nc.dram_tensor("a_vec", [2 * F_OUT, 1], F32, kind="ExternalInput")
    adjT = nc.dram_tensor("adjT", [N_O, R], BF16, kind="ExternalInput")
    out = nc.dram_tensor("out", [F_OUT, R], F32, kind="ExternalOutput")

    with tile.TileContext(nc) as tc, contextlib.ExitStack() as ctx:
        singles = ctx.enter_context(tc.tile_pool(name="singles", bufs=1))
        stage = ctx.enter_context(tc.tile_pool(name="stage", bufs=2))
        adj_pool = ctx.enter_context(tc.tile_pool(name="adj", bufs=10))
        x_pool = ctx.enter_context(tc.tile_pool(name="x", bufs=8))
        u_pool = ctx.enter_context(tc.tile_pool(name="u", bufs=5))
        v_pool = ctx.enter_context(tc.tile_pool(name="v", bufs=10))
        acc_psum = ctx.enter_context(tc.tile_pool(name="acc", bufs=1, space="PSUM"))
        ho_psum = ctx.enter_context(tc.tile_pool(name="hops", bufs=2, space="PSUM"))
        misc_psum = ctx.enter_context(tc.tile_pool(name="mpsum", bufs=2, space="PSUM"))

        # ------------------------------------------------------------------
        # Weights / t-side setup
        # ------------------------------------------------------------------
        wt_f = singles.tile([128, KC, F_OUT], F32)
        wo_f = singles.tile([128, KC, F_OUT], F32)
        for c in range(KC):
            nc.sync.dma_start(out=wt_f[:, c, :], in_=w_t[c * 128:(c + 1) * 128, :])
            nc.sync.dma_start(out=wo_f[:, c, :], in_=w_o[c * 128:(c + 1) * 128, :])
        a_t_sb = singles.tile([F_OUT, 1], F32)
        nc.sync.dma_start(out=a_t_sb[:, :], in_=a_vec[0:F_OUT, :])
        # a_o broadcast over 128 partitions (step-0 partition AP from DRAM)
        a_o_b = singles.tile([128, F_OUT], F32)
        for q in range(4):
            nc.sync.dma_start(
                out=a_o_b[q * 32:(q + 1) * 32, :],
                in_=bass.AP(tensor=a_vec, offset=F_OUT, ap=[[0, 32], [1, F_OUT]]),
            )
        # wo_ext bf16 [128, KC, 65]: projection weights plus an extra column
        # ao_eff = W_o @ a_o so the projection matmul also emits s_o = o.ao_eff
        wo_ext = singles.tile([128, KC, FE], BF16)
        nc.vector.tensor_copy(wo_ext[:, :, 0:F_OUT], wo_f[:, :, :])
        prod_ao = stage.tile([128, KC, F_OUT], F32, tag="prod_ao")
        a_o_rep = bass.AP(
            tensor=a_o_b[:, :].tensor,
            offset=a_o_b[:, :].offset,
            ap=[a_o_b[:, :].ap[0], [0, KC], [1, F_OUT]],
        )
        nc.vector.tensor_tensor(prod_ao[:, :, :], wo_f[:, :, :], a_o_rep, OP.mult)
        aoef = singles.tile([128, KC], F32)
        nc.vector.tensor_reduce(
            aoef[:, :], prod_ao[:, :, :], mybir.AxisListType.X, OP.add
        )
        nc.vector.tensor_copy(wo_ext[:, :, F_OUT], aoef[:, :])
        ones_row = singles.tile([1, F_OUT], F32)
        nc.vector.memset(ones_row[:, :], 1.0)

        t_b = singles.tile([128, KC, R], BF16)
        for c in range(KC):
            nc.sync.dma_start(out=t_b[:, c, :], in_=t_T[c * 128:(c + 1) * 128, :])

        # o_input.T resident (bf16); group 0 loaded first so its h_o setup can
        # start while the rest streams in during the main loop
        o_sb = singles.tile([128, KC, N_O], BF16)
        OG = GROUP * 128  # columns per group = 2048
        for h in range(2):
            for c in range(KC):
                nc.sync.dma_start(
                    out=o_sb[:, c, h * 1024:(h + 1) * 1024],
                    in_=o_T[c * 128:(c + 1) * 128, h * 1024:(h + 1) * 1024],
                )

        wt_b = singles.tile([128, KC, F_OUT], BF16)
        nc.vector.tensor_copy(wt_b[:, :, :], wt_f[:, :, :])
        # h_tT [F_OUT, R] -> s_t -> p = exp(0.8 s_t)
        ht_sb = singles.tile([F_OUT, R], F32)
        for n in range(NI2):
            ht_ps = misc_psum.tile([F_OUT, 512], F32, tag="mps")
            for c in range(KC):
                nc.tensor.matmul(
                    ht_ps[:, :],
                    wt_b[:, c, :],
                    t_b[:, c, n * 512:(n + 1) * 512],
                    start=(c == 0),
                    stop=(c == KC - 1),
                )
            nc.vector.tensor_copy(ht_sb[:, n * 512:(n + 1) * 512], ht_ps[:, :])

        p_row_b = singles.tile([1, R], BF16)
        for n in range(NI2):
            st_ps = misc_psum.tile([1, 512], F32, tag="mps")
            nc.tensor.matmul(
                st_ps[:, :],
                a_t_sb[:, :],
                ht_sb[:, n * 512:(n + 1) * 512],
                start=True,
                stop=True,
            )
            nc.scalar.activation(
                p_row_b[:, n * 512:(n + 1) * 512], st_ps[:, :], AF.Exp, scale=0.8
            )

        # p broadcast to all partitions [128, R] bf16 via DRAM bounce
        p_dram = nc.dram_tensor("p_bounce", [1, R], BF16, kind="Internal")
        nc.sync.dma_start(out=p_dram[:, :], in_=p_row_b[0:1, :])
        p_bcast = singles.tile([128, R], BF16)
        for q in range(8):
            nc.sync.dma_start(
                out=p_bcast[q * 16:(q + 1) * 16, :],
                in_=bass.AP(tensor=p_dram, offset=0, ap=[[0, 16], [1, R]]),
            )
        for g in range(1, NG):
            for c in range(KC):
                nc.sync.dma_start(
                    out=o_sb[:, c, g * OG:(g + 1) * OG],
                    in_=o_T[c * 128:(c + 1) * 128, g * OG:(g + 1) * OG],
                )


        # ------------------------------------------------------------------
        # o-side setup per group: hoext (bf16, ones col), v1/v2 (f32)
        # ------------------------------------------------------------------
        BATCH = 4  # j-tiles per psum batch ([128, 4, 65] f32 = 1040B < bank)
        hoext_tiles = [
            singles.tile([128, GROUP, FE], BF16, tag=f"hoext{g}", name=f"hoext{g}")
            for g in range(NG)
        ]
        v1_tiles = [
            singles.tile([128, GROUP], F32, tag=f"v1{g}", name=f"v1_{g}")
            for g in range(NG)
        ]
        v2_tiles = [
            singles.tile([128, GROUP], F32, tag=f"v2{g}", name=f"v2_{g}")
            for g in range(NG)
        ]
        negv2_tiles = [
            singles.tile([128, GROUP], F32, tag=f"nv2{g}", name=f"nv2_{g}")
            for g in range(NG)
        ]
        w2_tiles = [
            singles.tile([128, GROUP, FE], BF16, tag=f"w2{g}", name=f"w2_{g}")
            for g in range(NG)
        ]

        def emit_group_dma(g):
            for c in range(KC):
                nc.sync.dma_start(
                    out=o_sb[:, c, g * OG:(g + 1) * OG],
                    in_=o_T[c * 128:(c + 1) * 128, g * OG:(g + 1) * OG],
                )

        def emit_group_setup(g):
            hoext_g = hoext_tiles[g]
            v1_g, v2_g, negv2_g = v1_tiles[g], v2_tiles[g], negv2_tiles[g]
            so_g = stage.tile([128, GROUP], F32, tag="so")
            for u in range(0, GROUP, BATCH):
                ho_ps = ho_psum.tile([128, BATCH, FE], F32, tag="hops")
                for s in range(BATCH):
                    j0 = (g * GROUP + u + s) * 128
                    for c in range(KC):
                        nc.tensor.matmul(
                            ho_ps[:, s, :],
                            o_sb[:, c, j0:j0 + 128],
                            wo_ext[:, c, :],
                            start=(c == 0),
                            stop=(c == KC - 1),
                        )
                # h_o columns -> bf16 staging (scalar engine; idle in setup)
                nc.scalar.activation(
                    hoext_g[:, u:u + BATCH, 0:F_OUT], ho_ps[:, :, 0:F_OUT], AF.Copy
                )
                # s_o column -> f32
                nc.vector.tensor_copy(so_g[:, u:u + BATCH], ho_ps[:, :, F_OUT])
            # ones column for the softmax denominator
            nc.vector.memset(hoext_g[:, :, F_OUT], 1.0)
            nc.scalar.activation(v1_g[:, :], so_g[:, :], AF.Exp)
            nc.scalar.activation(v2_g[:, :], so_g[:, :], AF.Exp, scale=0.2)
            nc.vector.tensor_scalar_mul(negv2_g[:, :], v2_g[:, :], -1.0)
            # w2 = v2*hoext stationaries for 'a' tiles' Term1 (gpsimd)
            v2_rep = bass.AP(
                tensor=v2_g[:, :].tensor,
                offset=v2_g[:, :].offset,
                ap=[v2_g[:, :].ap[0], [1, GROUP], [0, FE]],
            )
            nc.gpsimd.tensor_tensor(
                w2_tiles[g][:, :, :], hoext_g[:, :, :], v2_rep, OP.mult
            )

        for _g in range(NG):
            emit_group_setup(_g)

        # ------------------------------------------------------------------
        # Main loop over j-tiles.
        #  'f': V = max(p v1, v2)*adj via fused DVE op; one matmul per chunk.
        #  'a': U = relu(p v1 - v2) on Scalar; V2 = U*adj; Term1 = w2^T adj
        #       and Term2 = hoext^T V2 both accumulate into acc.
        # ------------------------------------------------------------------
        acc = [
            acc_psum.tile([FE, 512], F32, tag=f"acc{n}", name=f"acc{n}")
            for n in range(NI2)
        ]
        total_mm = sum(2 if TILE_MODE[t] == 'a' else 1 for t in range(NJ))
        mm_idx = [0] * NI2

        def acc_flags(n):
            i = mm_idx[n]
            mm_idx[n] += 1
            return (i == 0), (i == total_mm - 1)

        LEAD = 4  # tiles of slack between V production and PE consumption
        pend = {}  # t -> (v_t, adj_t)

        def emit_elementwise(t):
            g, uu = divmod(t, GROUP)
            adj_t = adj_pool.tile([128, R], BF16)
            nc.sync.dma_start(out=adj_t[:, :], in_=adjT[t * 128:(t + 1) * 128, :])
            if TILE_MODE[t] == 'a':
                u_t = u_pool.tile([128, R], BF16)
                nc.scalar.activation(
                    u_t[:, :], p_bcast[:, :], AF.Relu,
                    bias=negv2_tiles[g][:, uu:uu + 1],
                    scale=v1_tiles[g][:, uu:uu + 1],
                )
                v_t = v_pool.tile([128, R], BF16)
                eng = nc.vector if MULT_ENG[t] == 'v' else nc.gpsimd
                eng.tensor_tensor(v_t[:, :], u_t[:, :], adj_t[:, :], OP.mult)
            else:
                x_t = x_pool.tile([128, R], BF16)
                nc.vector.tensor_scalar(
                    x_t[:, :], p_bcast[:, :],
                    v1_tiles[g][:, uu:uu + 1],
                    v2_tiles[g][:, uu:uu + 1],
                    OP.mult, OP.max,
                )
                v_t = v_pool.tile([128, R], BF16)
                nc.vector.tensor_tensor(v_t[:, :], x_t[:, :], adj_t[:, :], OP.mult)
            pend[t] = (v_t, adj_t)

        def emit_matmuls(t):
            g, uu = divmod(t, GROUP)
            v_t, adj_t = pend.pop(t)
            if TILE_MODE[t] == 'a':
                for n in range(NI2):
                    st, sp = acc_flags(n)
                    nc.tensor.matmul(
                        acc[n][:, :],
                        w2_tiles[g][:, uu, :],
                        adj_t[:, n * 512:(n + 1) * 512],
                        start=st, stop=sp,
                    )
            for n in range(NI2):
                st, sp = acc_flags(n)
                nc.tensor.matmul(
                    acc[n][:, :],
                    hoext_tiles[g][:, uu, :],
                    v_t[:, n * 512:(n + 1) * 512],
                    start=st, stop=sp,
                )

        for i in range(NJ + LEAD):
            if i < NJ:
                emit_elementwise(i)
            if i >= LEAD:
                emit_matmuls(i - LEAD)

        # ------------------------------------------------------------------
        # Epilogue: out = elu(Num[0:64] / Num[64])
        # ------------------------------------------------------------------
        h_sb = singles.tile([FE, R], F32)
        for n in range(NI2):
            nc.vector.tensor_copy(h_sb[:, n * 512:(n + 1) * 512], acc[n][:, :])
        den_dram = nc.dram_tensor("den_bounce", [1, R], F32, kind="Internal")
        nc.sync.dma_start(out=den_dram[:, :], in_=h_sb[F_OUT:FE, :])
        den128 = singles.tile([128, 8], F32)
        nc.sync.dma_start(
            out=den128[:, :],
            in_=bass.AP(tensor=den_dram, offset=0, ap=[[8, 128], [1, 8]]),
        )
        zr128 = singles.tile([128, 8], F32)
        nc.vector.reciprocal(zr128[:, :], den128[:, :])
        zr_dram = nc.dram_tensor("zr_bounce", [1, R], F32, kind="Internal")
        nc.sync.dma_start(
            out=bass.AP(tensor=zr_dram, offset=0, ap=[[8, 128], [1, 8]]),
            in_=zr128[:, :],
        )
        zb_sb = singles.tile([F_OUT, R], F32)
        nc.sync.dma_start(
            out=zb_sb[:, :],
            in_=bass.AP(tensor=zr_dram, offset=0, ap=[[0, F_OUT], [1, R]]),
        )
        ot_sb = singles.tile([F_OUT, R], F32)
        nc.vector.tensor_tensor(
            ot_sb[:, :], h_sb[0:F_OUT, :], zb_sb[:, :], OP.mult
        )

        # elu(x) = max(x,0) - 1 + exp(min(x,0))
        mn_sb = singles.tile([F_OUT, R], F32)
        ex_sb = singles.tile([F_OUT, R], F32)
        nc.vector.tensor_scalar(mn_sb[:, :], ot_sb[:, :], 0.0, None, OP.min)
        nc.scalar.activation(ex_sb[:, :], mn_sb[:, :], AF.Exp)
        nc.vector.tensor_scalar(ot_sb[:, :], ot_sb[:, :], 0.0, -1.0, OP.max, OP.add)
        nc.vector.tensor_tensor(ot_sb[:, :], ot_sb[:, :], ex_sb[:, :], OP.add)
        nc.sync.dma_start(out=out[:, :], in_=ot_sb[:, :])

    if split_waits:
        _split_multi_waits(nc)
    return nc


_CACHED = {}


def _get_compiled():
    if "nc" not in _CACHED:
        _CACHED["nc"] = build_kernel()
    return _CACHED["nc"]


def kernel(t_input, o_input, W_t, W_o, a, adj, _trace=False):
    from concourse.bass_utils import run_bass_kernel_spmd

    t_input = np.asarray(t_input, dtype=np.float32)
    o_input = np.asarray(o_input, dtype=np.float32)
    W_t = np.asarray(W_t, dtype=np.float32)
    W_o = np.asarray(W_o, dtype=np.float32)
    a = np.asarray(a, dtype=np.float32)
    adj = np.asarray(adj)

    o_T = np.ascontiguousarray(o_input.T.astype(bf16))
    adj_b = adj.astype(bf16)

    in_maps = []
    for m in range(N_CORES):
        rows = slice(m * R, (m + 1) * R)
        in_maps.append(
            {
                "t_T": np.ascontiguousarray(t_input[rows, :].T.astype(bf16)),
                "o_T": o_T,
                "w_t": W_t,
                "w_o": W_o,
                "a_vec": a,
                "adjT": np.ascontiguousarray(adj_b[rows, :].T),
            }
        )

    nc = _get_compiled()
    res = run_bass_kernel_spmd(
        nc, in_maps, core_ids=list(range(N_CORES)), trace=_trace
    )
    out = np.empty((N_T, F_OUT), dtype=np.float32)
    for m in range(N_CORES):
        out[m * R:(m + 1) * R, :] = res.results[m]["out"].T
    if _trace:
        kernel.last_exec_time_ns = res.exec_time_ns
        kernel.last_results = res
    return out
